# revision 8
# baseline (speedup 1.0000x reference)
"""Multi-head attention TRN2 kernel (B=4, S=2048, E=1024, H=16, D=64) on 8 cores.

Sharding: core c handles (batch b = c//2, query-half hq = c%2). Each core gets
the full batch-b sequence (rotated so its query half is rows 0-1023 -- softmax
over keys is order-invariant) and computes y rows for its 1024 queries. No
collectives; outputs concatenate.

Per-core dataflow (all matmuls in float32r -- full fp32 data, ~1e-3 matmul
precision at bf16-rate on the PE):
  1. Transpose x [2048,1024] -> xT [e,s] via PE-transpose (e on partitions).
  2. V = x @ Wv + bv  (s-major, [s, h*64+d]) -> DRAM scratch.
  3. Per head-pair p (heads 2p, 2p+1):
     QT_p [d2, 1024], KT_p [d2, 2048] (d2 = pair dims on partitions) from
     lhsT=W slices, rhs=xT.  Scores^T [k, q] per k-tile via row-tiled pair
     (K=64 each at array rows 0-63 / 64-127).  exp via ACT (scale=1/8) into
     PT [k, h0q|h1q].  attnV with ones-augmented V stationary [k,65]: rows
     0-63 = attn_outT, row 64 = softmax denominator -- accumulated over 16
     k-tiles in PSUM.  Normalize by broadcast reciprocal at eviction.
  4. y = attn_outT.T @ W_out + b_out per 128-row q-tile.
"""
from contextlib import ExitStack

import numpy as np

import concourse.bass as bass
import concourse.tile as tile
from concourse import bacc, mybir
from concourse.bass_utils import run_bass_kernel_spmd
from concourse.masks import make_identity

F32R = mybir.dt.float32r
F32 = mybir.dt.float32
AF = mybir.ActivationFunctionType

B, S, E, H, D = 4, 2048, 1024, 16, 64
Q = 1024          # queries per core
ET = 8            # e-tiles (contraction over E)
ST = 16           # s-tiles of the sequence
KT = 16           # k-tiles in attention
NP = 8            # head-pairs
N_CORES = 8


def _bcast_dram(ap1d, n_part, n_free):
    """Broadcast a DRAM row across n_part partitions: [[0,n_part],[1,n_free]]."""
    return bass.AP(
        tensor=ap1d.tensor, offset=ap1d.offset, ap=[[0, n_part], [1, n_free]]
    )


def _bcast_free(src, counts):
    """Broadcast an SBUF [P,1] AP along extra zero-stride free dims."""
    return bass.AP(
        tensor=src.tensor,
        offset=src.offset,
        ap=[list(src.ap[0])] + [[0, c] for c in counts],
    )


def _bcast_row(src_row, n_part, n_free):
    """Broadcast one SBUF partition row across n_part partitions."""
    return bass.AP(
        tensor=src_row.tensor, offset=src_row.offset, ap=[[0, n_part], [1, n_free]]
    )


def _emit(tc, nc, x, wqkv, bqkv, wout, bout, y, vdram, rscr):
    with ExitStack() as ctx:
        xt_pool = ctx.enter_context(tc.tile_pool(name="xt", bufs=1))
        const = ctx.enter_context(tc.tile_pool(name="const", bufs=1))

        xt = xt_pool.tile([128, ET, S], F32R)

        ident = const.tile([128, 128], F32)
        make_identity(nc, ident)
        bqk_t = const.tile([128, 24], F32)
        nc.sync.dma_start(out=bqk_t, in_=bqkv.rearrange("(j p) -> p j", p=128).bitcast(F32))
        bv_t = const.tile([128, E], F32R)
        nc.sync.dma_start(out=bv_t, in_=_bcast_dram(bqkv[2 * E : 2 * E + 1], 128, E))
        bout_t = const.tile([128, E], F32)
        nc.sync.dma_start(out=bout_t, in_=_bcast_dram(bout[0:1], 128, E))
        ones_f = const.tile([128, 1], F32)
        nc.vector.memset(ones_f, 1.0)
        ones_r = const.tile([128, 1], F32R)
        nc.vector.tensor_copy(ones_r, ones_f)

        # ---- phase 0: x -> xT ----
        with (
            tc.tile_pool(name="xload", bufs=3) as xload,
            tc.tile_pool(name="tps", bufs=4, space="PSUM") as tps,
        ):
            for st in range(ST):
                xs = xload.tile([128, E], F32)
                nc.sync.dma_start(out=xs, in_=x[st * 128 : (st + 1) * 128, :].bitcast(F32))
                for g in range(2):
                    ps = tps.tile([128, 4, 128], F32)
                    for i in range(4):
                        et = g * 4 + i
                        nc.tensor.transpose(
                            ps[:, i, :], xs[:, et * 128 : (et + 1) * 128], ident
                        )
                    nc.vector.tensor_copy(
                        xt[:, g * 4 : (g + 1) * 4, st * 128 : (st + 1) * 128], ps
                    )

        # ---- phase A: V = x @ Wv + bv -> vdram ----
        with (
            tc.tile_pool(name="wv", bufs=1) as wvp,
            tc.tile_pool(name="vps", bufs=4, space="PSUM") as vps,
            tc.tile_pool(name="vev", bufs=3) as vev,
        ):
            wv = wvp.tile([128, ET, E], F32R)
            nc.sync.dma_start(
                out=wv, in_=wqkv[:, 2 * E : 3 * E].rearrange("(t p) n -> p t n", p=128)
            )
            for st in range(ST):
                ps = vps.tile([128, E], F32)
                for half in range(2):
                    for et in range(ET):
                        nc.tensor.matmul(
                            ps[:, half * 512 : (half + 1) * 512],
                            xt[:, et, st * 128 : (st + 1) * 128],
                            wv[:, et, half * 512 : (half + 1) * 512],
                            start=(et == 0),
                            stop=(et == ET - 1),
                        )
                vb = vev.tile([128, E], F32R)
                nc.vector.tensor_add(vb, ps, bv_t)
                nc.sync.dma_start(out=vdram[st * 128 : (st + 1) * 128, :], in_=vb)

        # ---- phases B/C: per-pair QK + attention; then out-proj ----
        aout_pool = ctx.enter_context(tc.tile_pool(name="aout", bufs=1))
        aout = aout_pool.tile([128, NP, Q], F32R)

        with (
            tc.tile_pool(name="wqk", bufs=2) as wqkp,
            tc.tile_pool(name="qt", bufs=2) as qtp,
            tc.tile_pool(name="kt", bufs=2) as ktp,
            tc.tile_pool(name="vp", bufs=2) as vpp,
            tc.tile_pool(name="pt", bufs=3) as ptp,
            tc.tile_pool(name="ev", bufs=1) as evp,
            tc.tile_pool(name="qkps", bufs=1, space="PSUM") as qkps,
            tc.tile_pool(name="scps", bufs=2, space="PSUM") as scps,
            tc.tile_pool(name="accps", bufs=2, space="PSUM") as accps,
        ):
            for p in range(NP):
                wq = wqkp.tile([128, ET, 128], F32R, tag="wq")
                nc.sync.dma_start(
                    out=wq,
                    in_=wqkv[:, p * 128 : (p + 1) * 128].rearrange(
                        "(t p2) m -> p2 t m", p2=128
                    ),
                )
                wk = wqkp.tile([128, ET, 128], F32R, tag="wk")
                nc.sync.dma_start(
                    out=wk,
                    in_=wqkv[:, E + p * 128 : E + (p + 1) * 128].rearrange(
                        "(t p2) m -> p2 t m", p2=128
                    ),
                )

                qt_t = qtp.tile([128, Q], F32R)
                ps = qkps.tile([128, 1024], F32, tag="qk")
                for half in range(2):
                    for et in range(ET):
                        nc.tensor.matmul(
                            ps[:, half * 512 : (half + 1) * 512],
                            wq[:, et, :],
                            xt[:, et, half * 512 : (half + 1) * 512],
                            start=(et == 0),
                            stop=(et == ET - 1),
                        )
                nc.vector.tensor_scalar_add(qt_t, ps, bqk_t[:, p : p + 1])

                kt_t = ktp.tile([128, S], F32R)
                for kh in range(2):
                    ps2 = qkps.tile([128, 1024], F32, tag="qk")
                    for half in range(2):
                        for et in range(ET):
                            nc.tensor.matmul(
                                ps2[:, half * 512 : (half + 1) * 512],
                                wk[:, et, :],
                                xt[:, et, kh * 1024 + half * 512 : kh * 1024 + (half + 1) * 512],
                                start=(et == 0),
                                stop=(et == ET - 1),
                            )
                    nc.vector.tensor_scalar_add(
                        kt_t[:, kh * 1024 : (kh + 1) * 1024], ps2,
                        bqk_t[:, 8 + p : 9 + p],
                    )

                vp = vpp.tile([128, KT, 2, 65], F32R)
                for h in range(2):
                    nc.sync.dma_start(
                        out=vp[:, :, h, 0:64],
                        in_=vdram[
                            :, p * 128 + h * 64 : p * 128 + h * 64 + 64
                        ].rearrange("(t p2) d -> p2 t d", p2=128),
                    )
                nc.sync.dma_start(out=vp[:, :, :, 64:65], in_=_bcast_free(ones_r[:, 0:1], [KT * 2]))

                for qh in range(2):
                    qsl = slice(qh * 512, (qh + 1) * 512)
                    acc0 = accps.tile([128, 512], F32, tag="acc")
                    acc1 = accps.tile([128, 512], F32, tag="acc")
                    for k in range(KT):
                        sc = scps.tile([128, 1024], F32, tag="sc")
                        nc.tensor.matmul(
                            sc[:, 0:512],
                            kt_t[0:64, k * 128 : (k + 1) * 128],
                            qt_t[0:64, qsl],
                            start=True, stop=True,
                        )
                        nc.tensor.matmul(
                            sc[:, 512:1024],
                            kt_t[64:128, k * 128 : (k + 1) * 128],
                            qt_t[64:128, qsl],
                            start=True, stop=True,
                        )
                        pt_t = ptp.tile([128, 1024], F32R)
                        nc.scalar.activation(out=pt_t, in_=sc, func=AF.Exp, scale=0.125)
                        nc.tensor.matmul(
                            acc0[0:65, :], vp[:, k, 0, :], pt_t[:, 0:512],
                            start=(k == 0), stop=(k == KT - 1),
                        )
                        nc.tensor.matmul(
                            acc1[0:65, :], vp[:, k, 1, :], pt_t[:, 512:1024],
                            start=(k == 0), stop=(k == KT - 1),
                        )
                    # eviction: normalize by 1/rowsum
                    rs = evp.tile([128, 1024], F32, tag="rs")
                    nc.vector.reciprocal(rs[64:65, 0:512], acc0[64:65, :])
                    nc.vector.reciprocal(rs[64:65, 512:1024], acc1[64:65, :])
                    ridx = p * 2 + qh
                    nc.sync.dma_start(out=rscr[ridx : ridx + 1, :], in_=rs[64:65, :])
                    sc0 = evp.tile([64, 512], F32, tag="sc0")
                    nc.sync.dma_start(out=sc0, in_=_bcast_dram(rscr[ridx, 0:1], 64, 512))
                    sc1 = evp.tile([64, 512], F32, tag="sc1")
                    nc.sync.dma_start(out=sc1, in_=_bcast_dram(rscr[ridx, 512:513], 64, 512))
                    nc.vector.tensor_mul(aout[0:64, p, qsl], acc0[0:64, :], sc0)
                    tmp1 = evp.tile([64, 512], F32R, tag="tmp1")
                    nc.vector.tensor_mul(tmp1, acc1[0:64, :], sc1)
                    nc.sync.dma_start(out=aout[64:128, p, qsl], in_=tmp1)

        # ---- phase C: y = attn_out @ W_out + b_out ----
        with (
            tc.tile_pool(name="wo", bufs=1) as wop,
            tc.tile_pool(name="yps", bufs=4, space="PSUM") as yps,
            tc.tile_pool(name="yev", bufs=3) as yev,
        ):
            wo = wop.tile([128, ET, E], F32R)
            nc.sync.dma_start(out=wo, in_=wout.rearrange("(t p) n -> p t n", p=128))
            for qt_i in range(8):
                ps = yps.tile([128, E], F32)
                for half in range(2):
                    for p8 in range(8):
                        nc.tensor.matmul(
                            ps[:, half * 512 : (half + 1) * 512],
                            aout[:, p8, qt_i * 128 : (qt_i + 1) * 128],
                            wo[:, p8, half * 512 : (half + 1) * 512],
                            start=(p8 == 0),
                            stop=(p8 == 7),
                        )
                yb = yev.tile([128, E], F32)
                nc.vector.tensor_add(yb, ps, bout_t)
                nc.sync.dma_start(out=y[qt_i * 128 : (qt_i + 1) * 128, :], in_=yb)


def build_nc():
    nc = bacc.Bacc("TRN2", target_bir_lowering=False, debug=False)
    x = nc.dram_tensor("x", [S, E], F32R, kind="ExternalInput").ap()
    wqkv = nc.dram_tensor("wqkv", [E, 3 * E], F32R, kind="ExternalInput").ap()
    bqkv = nc.dram_tensor("bqkv", [3 * E], F32R, kind="ExternalInput").ap()
    wout = nc.dram_tensor("wout", [E, E], F32R, kind="ExternalInput").ap()
    bout = nc.dram_tensor("bout", [E], F32, kind="ExternalInput").ap()
    y = nc.dram_tensor("y", [Q, E], F32, kind="ExternalOutput").ap()
    vdram = nc.dram_tensor("vdram", [S, E], F32R).ap()
    rscr = nc.dram_tensor("rscr", [16, 1024], F32).ap()
    with tile.TileContext(nc) as tc:
        _emit(tc, nc, x, wqkv, bqkv, wout, bout, y, vdram, rscr)
    nc.compile()
    return nc


_NC = None


def _get_nc():
    global _NC
    if _NC is None:
        _NC = build_nc()
    return _NC


def make_in_maps(x, W_qkv, b_qkv, W_out, b_out):
    x = np.ascontiguousarray(np.asarray(x, dtype=np.float32))
    W_qkv = np.ascontiguousarray(np.asarray(W_qkv, dtype=np.float32))
    b_qkv = np.ascontiguousarray(np.asarray(b_qkv, dtype=np.float32))
    W_out = np.ascontiguousarray(np.asarray(W_out, dtype=np.float32))
    b_out = np.ascontiguousarray(np.asarray(b_out, dtype=np.float32))
    in_maps = []
    for c in range(N_CORES):
        b, hq = c // 2, c % 2
        xb = x[b]
        if hq:
            xb = np.ascontiguousarray(np.concatenate([xb[1024:], xb[:1024]], axis=0))
        in_maps.append(
            {"x": xb, "wqkv": W_qkv, "bqkv": b_qkv, "wout": W_out, "bout": b_out}
        )
    return in_maps


def assemble(results):
    out = np.empty((B, S, E), dtype=np.float32)
    for c in range(N_CORES):
        b, hq = c // 2, c % 2
        out[b, hq * 1024 : (hq + 1) * 1024, :] = results[c]["y"]
    return out


def kernel(x, W_qkv, b_qkv, W_out, b_out):
    nc = _get_nc()
    in_maps = make_in_maps(x, W_qkv, b_qkv, W_out, b_out)
    res = run_bass_kernel_spmd(nc, in_maps, list(range(N_CORES)))
    return assemble(res.results)


# revision 13
# speedup vs baseline: 1.0153x; 1.0153x over previous
"""Multi-head attention TRN2 kernel (B=4, S=2048, E=1024, H=16, D=64) on 8 cores.

Sharding: core c handles (batch b = c//2, query-half hq = c%2). Each core gets
the full batch-b sequence (rotated so its query half is rows 0-1023 -- softmax
over keys is order-invariant) and computes y rows for its 1024 queries. No
collectives; outputs concatenate.

Per-core dataflow (all matmuls in float32r -- full fp32 data, ~1e-3 matmul
precision at bf16-rate on the PE):
  1. Transpose x [2048,1024] -> xT [e,s] via PE-transpose (e on partitions).
  2. V = x @ Wv + bv  (s-major, [s, h*64+d]) -> DRAM scratch.
  3. Per head-pair p (heads 2p, 2p+1):
     QT_p [d2, 1024], KT_p [d2, 2048] (d2 = pair dims on partitions) from
     lhsT=W slices, rhs=xT.  Scores^T [k, q] per k-tile via row-tiled pair
     (K=64 each at array rows 0-63 / 64-127).  exp via ACT (scale=1/8) into
     PT [k, h0q|h1q].  attnV with ones-augmented V stationary [k,65]: rows
     0-63 = attn_outT, row 64 = softmax denominator -- accumulated over 16
     k-tiles in PSUM.  Normalize by broadcast reciprocal at eviction.
  4. y = attn_outT.T @ W_out + b_out per 128-row q-tile.
"""
from contextlib import ExitStack

import numpy as np

import concourse.bass as bass
import concourse.tile as tile
from concourse import bacc, mybir
from concourse.bass_utils import run_bass_kernel_spmd
from concourse.masks import make_identity

F32R = mybir.dt.float32r
F32 = mybir.dt.float32
AF = mybir.ActivationFunctionType

B, S, E, H, D = 4, 2048, 1024, 16, 64
Q = 1024          # queries per core
ET = 8            # e-tiles (contraction over E)
ST = 16           # s-tiles of the sequence
KT = 16           # k-tiles in attention
NP = 8            # head-pairs
N_CORES = 8


def _bcast_dram(ap1d, n_part, n_free):
    """Broadcast a DRAM row across n_part partitions: [[0,n_part],[1,n_free]]."""
    return bass.AP(
        tensor=ap1d.tensor, offset=ap1d.offset, ap=[[0, n_part], [1, n_free]]
    )


def _bcast_free(src, counts):
    """Broadcast an SBUF [P,1] AP along extra zero-stride free dims."""
    return bass.AP(
        tensor=src.tensor,
        offset=src.offset,
        ap=[list(src.ap[0])] + [[0, c] for c in counts],
    )


def _bcast_row(src_row, n_part, n_free):
    """Broadcast one SBUF partition row across n_part partitions."""
    return bass.AP(
        tensor=src_row.tensor, offset=src_row.offset, ap=[[0, n_part], [1, n_free]]
    )


def _emit(tc, nc, x, wqkv, bqkv, wout, bout, y, vdram, rscr):
    with ExitStack() as ctx:
        xt_pool = ctx.enter_context(tc.tile_pool(name="xt", bufs=1))
        const = ctx.enter_context(tc.tile_pool(name="const", bufs=1))

        xt = xt_pool.tile([128, ET, S], F32R)

        ident = const.tile([128, 128], F32)
        make_identity(nc, ident)
        bqk_t = const.tile([128, 24], F32)
        nc.sync.dma_start(out=bqk_t, in_=bqkv.rearrange("(j p) -> p j", p=128).bitcast(F32))
        bv_t = const.tile([128, E], F32R)
        nc.sync.dma_start(out=bv_t, in_=_bcast_dram(bqkv[2 * E : 2 * E + 1], 128, E))
        bout_t = const.tile([128, E], F32)
        nc.sync.dma_start(out=bout_t, in_=_bcast_dram(bout[0:1], 128, E))
        ones_f = const.tile([128, 1], F32)
        nc.vector.memset(ones_f, 1.0)
        ones_r = const.tile([128, 1], F32R)
        nc.vector.tensor_copy(ones_r, ones_f)

        # ---- phase 0: x -> xT (wv prefetched under the transposes) ----
        wv_ctx = tc.tile_pool(name="wv", bufs=1)
        wvp = wv_ctx.__enter__()
        wv = wvp.tile([128, ET, E], F32R)
        nc.sync.dma_start(
            out=wv, in_=wqkv[:, 2 * E : 3 * E].rearrange("(t p) n -> p t n", p=128)
        )
        with (
            tc.tile_pool(name="xload", bufs=3) as xload,
            tc.tile_pool(name="tps", bufs=4, space="PSUM") as tps,
        ):
            for st in range(ST):
                xs = xload.tile([128, E], F32)
                nc.sync.dma_start(out=xs, in_=x[st * 128 : (st + 1) * 128, :].bitcast(F32))
                for g in range(2):
                    ps = tps.tile([128, 4, 128], F32)
                    for i in range(4):
                        et = g * 4 + i
                        nc.tensor.transpose(
                            ps[:, i, :], xs[:, et * 128 : (et + 1) * 128], ident
                        )
                    nc.vector.tensor_copy(
                        xt[:, g * 4 : (g + 1) * 4, st * 128 : (st + 1) * 128], ps
                    )

        # ---- phase A: V = x @ Wv + bv -> vdram ----
        with (
            tc.tile_pool(name="vps", bufs=4, space="PSUM") as vps,
            tc.tile_pool(name="vev", bufs=3) as vev,
        ):
            for st in range(ST):
                ps = vps.tile([128, E], F32)
                for half in range(2):
                    for et in range(ET):
                        nc.tensor.matmul(
                            ps[:, half * 512 : (half + 1) * 512],
                            xt[:, et, st * 128 : (st + 1) * 128],
                            wv[:, et, half * 512 : (half + 1) * 512],
                            start=(et == 0),
                            stop=(et == ET - 1),
                        )
                vb = vev.tile([128, E], F32R)
                nc.vector.tensor_add(vb, ps, bv_t)
                nc.sync.dma_start(out=vdram[st * 128 : (st + 1) * 128, :], in_=vb)
        wv_ctx.__exit__(None, None, None)

        # ---- phase B: per-pair QK JIT + attention, software-pipelined ----
        aout_pool = ctx.enter_context(tc.tile_pool(name="aout", bufs=1))
        aout = aout_pool.tile([128, NP, Q], F32R)

        with (
            tc.tile_pool(name="wqk", bufs=2) as wqkp,
            tc.tile_pool(name="qt", bufs=2) as qtp,
            tc.tile_pool(name="kt", bufs=2) as ktp,
            tc.tile_pool(name="vp", bufs=2) as vpp,
            tc.tile_pool(name="pt", bufs=3) as ptp,
            tc.tile_pool(name="ev", bufs=1) as evp,
            tc.tile_pool(name="qkps", bufs=1, space="PSUM") as qkps,
            tc.tile_pool(name="scps", bufs=2, space="PSUM") as scps,
            tc.tile_pool(name="accps", bufs=2, space="PSUM") as accps,
        ):

            def build_pair(p):
                """Allocate pair-p input tiles; return (tiles, emission thunks)."""
                wq = wqkp.tile([128, ET, 128], F32R, tag="wq")
                wk = wqkp.tile([128, ET, 128], F32R, tag="wk")
                qt_t = qtp.tile([128, Q], F32R)
                kt_t = ktp.tile([128, S], F32R)
                vp = vpp.tile([128, KT, 2, 65], F32R)
                th = []
                th.append(lambda: nc.sync.dma_start(
                    out=wq,
                    in_=wqkv[:, p * 128 : (p + 1) * 128].rearrange(
                        "(t p2) m -> p2 t m", p2=128),
                ))
                th.append(lambda: nc.sync.dma_start(
                    out=wk,
                    in_=wqkv[:, E + p * 128 : E + (p + 1) * 128].rearrange(
                        "(t p2) m -> p2 t m", p2=128),
                ))
                for h in range(2):
                    th.append(lambda h=h: nc.sync.dma_start(
                        out=vp[:, :, h, 0:64],
                        in_=vdram[
                            :, p * 128 + h * 64 : p * 128 + h * 64 + 64
                        ].rearrange("(t p2) d -> p2 t d", p2=128),
                    ))
                th.append(lambda: nc.sync.dma_start(
                    out=vp[:, :, :, 64:65], in_=_bcast_free(ones_r[:, 0:1], [KT * 2])
                ))

                def qk_group(dst, w, bias_col, xoff):
                    g = []
                    ps_box = []

                    def alloc():
                        qk_ps = qkps.tile([128, 1024], F32, tag="qk")
                        ps_box.append(qk_ps)
                    g.append(alloc)
                    for half in range(2):
                        for et in range(ET):
                            g.append(lambda half=half, et=et: nc.tensor.matmul(
                                ps_box[0][:, half * 512 : (half + 1) * 512],
                                w[:, et, :],
                                xt[:, et, xoff + half * 512 : xoff + (half + 1) * 512],
                                start=(et == 0),
                                stop=(et == ET - 1),
                            ))
                    g.append(lambda: nc.vector.tensor_scalar_add(dst, ps_box[0], bias_col))
                    return g

                th += qk_group(qt_t, wq, bqk_t[:, p : p + 1], 0)
                th += qk_group(kt_t[:, 0:1024], wk, bqk_t[:, 8 + p : 9 + p], 0)
                th += qk_group(kt_t[:, 1024:2048], wk, bqk_t[:, 8 + p : 9 + p], 1024)
                return {"qt": qt_t, "kt": kt_t, "vp": vp}, th

            cur, th0 = build_pair(0)
            for t in th0:
                t()

            for p in range(NP):
                if p + 1 < NP:
                    nxt, pending = build_pair(p + 1)
                else:
                    nxt = None
                    pending = []
                pending = list(pending)
                qt_t, kt_t, vp = cur["qt"], cur["kt"], cur["vp"]
                for qh in range(2):
                    qsl = slice(qh * 512, (qh + 1) * 512)
                    acc0 = accps.tile([128, 512], F32, tag="acc")
                    acc1 = accps.tile([128, 512], F32, tag="acc")
                    pts = [None] * KT
                    for k in range(KT):
                        sc = scps.tile([128, 1024], F32, tag="sc")
                        nc.tensor.matmul(
                            sc[:, 0:512],
                            kt_t[0:64, k * 128 : (k + 1) * 128],
                            qt_t[0:64, qsl],
                            start=True, stop=True,
                        )
                        nc.tensor.matmul(
                            sc[:, 512:1024],
                            kt_t[64:128, k * 128 : (k + 1) * 128],
                            qt_t[64:128, qsl],
                            start=True, stop=True,
                        )
                        if k >= 1:
                            pt_p = pts[k - 1]
                            nc.tensor.matmul(
                                acc0[0:65, :], vp[:, k - 1, 0, :], pt_p[:, 0:512],
                                start=(k - 1 == 0), stop=(k - 1 == KT - 1),
                            )
                            nc.tensor.matmul(
                                acc1[0:65, :], vp[:, k - 1, 1, :], pt_p[:, 512:1024],
                                start=(k - 1 == 0), stop=(k - 1 == KT - 1),
                            )
                        pt_t = ptp.tile([128, 1024], F32R)
                        pts[k] = pt_t
                        nc.scalar.activation(out=pt_t, in_=sc, func=AF.Exp, scale=0.125)
                        for _ in range(2):
                            if pending:
                                pending.pop(0)()
                    nc.tensor.matmul(
                        acc0[0:65, :], vp[:, KT - 1, 0, :], pts[KT - 1][:, 0:512],
                        start=False, stop=True,
                    )
                    nc.tensor.matmul(
                        acc1[0:65, :], vp[:, KT - 1, 1, :], pts[KT - 1][:, 512:1024],
                        start=False, stop=True,
                    )
                    # eviction: normalize by 1/rowsum (approx recip, 2 ULP)
                    rs = evp.tile([128, 1024], F32, tag="rs")
                    nc.vector.reciprocal(rs[64:65, 0:512], acc0[64:65, :])
                    nc.vector.reciprocal(rs[64:65, 512:1024], acc1[64:65, :])
                    ridx = p * 2 + qh
                    nc.sync.dma_start(out=rscr[ridx : ridx + 1, :], in_=rs[64:65, :])
                    sc0 = evp.tile([64, 512], F32, tag="sc0")
                    nc.sync.dma_start(out=sc0, in_=_bcast_dram(rscr[ridx, 0:1], 64, 512))
                    sc1 = evp.tile([64, 512], F32, tag="sc1")
                    nc.sync.dma_start(out=sc1, in_=_bcast_dram(rscr[ridx, 512:513], 64, 512))
                    nc.vector.tensor_mul(aout[0:64, p, qsl], acc0[0:64, :], sc0)
                    tmp1 = evp.tile([64, 512], F32R, tag="tmp1")
                    nc.vector.tensor_mul(tmp1, acc1[0:64, :], sc1)
                    nc.sync.dma_start(out=aout[64:128, p, qsl], in_=tmp1)
                for t in pending:
                    t()
                cur = nxt

        # ---- phase C: y = attn_out @ W_out + b_out ----
        with (
            tc.tile_pool(name="wo", bufs=2) as wop,
            tc.tile_pool(name="yps", bufs=4, space="PSUM") as yps,
            tc.tile_pool(name="yev", bufs=3) as yev,
        ):
            wo = [wop.tile([128, ET, 512], F32R, name=f"wo{h}", tag=f"wo{h}") for h in range(2)]
            for half in range(2):
                nc.sync.dma_start(
                    out=wo[half],
                    in_=wout[:, half * 512 : (half + 1) * 512].rearrange(
                        "(t p) n -> p t n", p=128
                    ),
                )
            for half in range(2):
                for qt_i in range(8):
                    ps = yps.tile([128, 512], F32)
                    for p8 in range(8):
                        nc.tensor.matmul(
                            ps,
                            aout[:, p8, qt_i * 128 : (qt_i + 1) * 128],
                            wo[half][:, p8, :],
                            start=(p8 == 0),
                            stop=(p8 == 7),
                        )
                    yb = yev.tile([128, 512], F32)
                    nc.vector.tensor_add(
                        yb, ps, bout_t[:, half * 512 : (half + 1) * 512]
                    )
                    nc.sync.dma_start(
                        out=y[qt_i * 128 : (qt_i + 1) * 128, half * 512 : (half + 1) * 512],
                        in_=yb,
                    )


def build_nc():
    nc = bacc.Bacc("TRN2", target_bir_lowering=False, debug=False)
    x = nc.dram_tensor("x", [S, E], F32R, kind="ExternalInput").ap()
    wqkv = nc.dram_tensor("wqkv", [E, 3 * E], F32R, kind="ExternalInput").ap()
    bqkv = nc.dram_tensor("bqkv", [3 * E], F32R, kind="ExternalInput").ap()
    wout = nc.dram_tensor("wout", [E, E], F32R, kind="ExternalInput").ap()
    bout = nc.dram_tensor("bout", [E], F32, kind="ExternalInput").ap()
    y = nc.dram_tensor("y", [Q, E], F32, kind="ExternalOutput").ap()
    vdram = nc.dram_tensor("vdram", [S, E], F32R).ap()
    rscr = nc.dram_tensor("rscr", [16, 1024], F32).ap()
    with tile.TileContext(nc) as tc:
        _emit(tc, nc, x, wqkv, bqkv, wout, bout, y, vdram, rscr)
    nc.compile()
    return nc


_NC = None


def _get_nc():
    global _NC
    if _NC is None:
        _NC = build_nc()
    return _NC


def make_in_maps(x, W_qkv, b_qkv, W_out, b_out):
    x = np.ascontiguousarray(np.asarray(x, dtype=np.float32))
    W_qkv = np.ascontiguousarray(np.asarray(W_qkv, dtype=np.float32))
    b_qkv = np.ascontiguousarray(np.asarray(b_qkv, dtype=np.float32))
    W_out = np.ascontiguousarray(np.asarray(W_out, dtype=np.float32))
    b_out = np.ascontiguousarray(np.asarray(b_out, dtype=np.float32))
    in_maps = []
    for c in range(N_CORES):
        b, hq = c // 2, c % 2
        xb = x[b]
        if hq:
            xb = np.ascontiguousarray(np.concatenate([xb[1024:], xb[:1024]], axis=0))
        in_maps.append(
            {"x": xb, "wqkv": W_qkv, "bqkv": b_qkv, "wout": W_out, "bout": b_out}
        )
    return in_maps


def assemble(results):
    out = np.empty((B, S, E), dtype=np.float32)
    for c in range(N_CORES):
        b, hq = c // 2, c % 2
        out[b, hq * 1024 : (hq + 1) * 1024, :] = results[c]["y"]
    return out


def kernel(x, W_qkv, b_qkv, W_out, b_out):
    nc = _get_nc()
    in_maps = make_in_maps(x, W_qkv, b_qkv, W_out, b_out)
    res = run_bass_kernel_spmd(nc, in_maps, list(range(N_CORES)))
    return assemble(res.results)


# revision 14
# speedup vs baseline: 1.2717x; 1.2525x over previous
"""Multi-head attention TRN2 kernel (B=4, S=2048, E=1024, H=16, D=64) on 8 cores.

Sharding: core c handles (batch b = c//2, query-half hq = c%2). Each core gets
the full batch-b sequence (rotated so its query half is rows 0-1023 -- softmax
over keys is order-invariant) and computes y rows for its 1024 queries. No
collectives; outputs concatenate.

Per-core dataflow (all matmuls in float32r -- full fp32 data, ~1e-3 matmul
precision at bf16-rate on the PE):
  1. Transpose x [2048,1024] -> xT [e,s] via PE-transpose (e on partitions).
  2. V = x @ Wv + bv  (s-major, [s, h*64+d]) -> DRAM scratch.
  3. Per head-pair p (heads 2p, 2p+1):
     QT_p [d2, 1024], KT_p [d2, 2048] (d2 = pair dims on partitions) from
     lhsT=W slices, rhs=xT.  Scores^T [k, q] per k-tile via row-tiled pair
     (K=64 each at array rows 0-63 / 64-127).  exp via ACT (scale=1/8) into
     PT [k, h0q|h1q].  attnV with ones-augmented V stationary [k,65]: rows
     0-63 = attn_outT, row 64 = softmax denominator -- accumulated over 16
     k-tiles in PSUM.  Normalize by broadcast reciprocal at eviction.
  4. y = attn_outT.T @ W_out + b_out per 128-row q-tile.
"""
from contextlib import ExitStack

import numpy as np

import concourse.bass as bass
import concourse.tile as tile
from concourse import bacc, mybir
from concourse.bass_utils import run_bass_kernel_spmd
from concourse.masks import make_identity

F32R = mybir.dt.float32r
F32 = mybir.dt.float32
AF = mybir.ActivationFunctionType

B, S, E, H, D = 4, 2048, 1024, 16, 64
Q = 1024          # queries per core
ET = 8            # e-tiles (contraction over E)
ST = 16           # s-tiles of the sequence
KT = 16           # k-tiles in attention
NP = 8            # head-pairs
N_CORES = 8


def _bcast_dram(ap1d, n_part, n_free):
    """Broadcast a DRAM row across n_part partitions: [[0,n_part],[1,n_free]]."""
    return bass.AP(
        tensor=ap1d.tensor, offset=ap1d.offset, ap=[[0, n_part], [1, n_free]]
    )


def _bcast_free(src, counts):
    """Broadcast an SBUF [P,1] AP along extra zero-stride free dims."""
    return bass.AP(
        tensor=src.tensor,
        offset=src.offset,
        ap=[list(src.ap[0])] + [[0, c] for c in counts],
    )


def _bcast_row(src_row, n_part, n_free):
    """Broadcast one SBUF partition row across n_part partitions."""
    return bass.AP(
        tensor=src_row.tensor, offset=src_row.offset, ap=[[0, n_part], [1, n_free]]
    )


def _emit(tc, nc, x, wqkv, bqkv, wout, bout, y, vdram, rscr, rscr2):
    with ExitStack() as ctx:
        xt_pool = ctx.enter_context(tc.tile_pool(name="xt", bufs=1))
        const = ctx.enter_context(tc.tile_pool(name="const", bufs=1))

        xt = xt_pool.tile([128, ET, S], F32R)

        ident = const.tile([128, 128], F32)
        make_identity(nc, ident)
        bqk_t = const.tile([128, 24], F32)
        nc.sync.dma_start(out=bqk_t, in_=bqkv.rearrange("(j p) -> p j", p=128).bitcast(F32))
        bv_t = const.tile([128, E], F32R)
        nc.sync.dma_start(out=bv_t, in_=_bcast_dram(bqkv[2 * E : 2 * E + 1], 128, E))
        bout_t = const.tile([128, E], F32)
        nc.sync.dma_start(out=bout_t, in_=_bcast_dram(bout[0:1], 128, E))
        ones_f = const.tile([128, 1], F32)
        nc.vector.memset(ones_f, 1.0)
        ones_r = const.tile([128, 1], F32R)
        nc.vector.tensor_copy(ones_r, ones_f)

        # ---- phase 0: x -> xT (wv prefetched under the transposes) ----
        wv_ctx = tc.tile_pool(name="wv", bufs=1)
        wvp = wv_ctx.__enter__()
        wv = wvp.tile([128, ET, E], F32R)
        nc.sync.dma_start(
            out=wv, in_=wqkv[:, 2 * E : 3 * E].rearrange("(t p) n -> p t n", p=128)
        )
        with (
            tc.tile_pool(name="xload", bufs=3) as xload,
            tc.tile_pool(name="tps", bufs=4, space="PSUM") as tps,
        ):
            for st in range(ST):
                xs = xload.tile([128, E], F32)
                nc.sync.dma_start(out=xs, in_=x[st * 128 : (st + 1) * 128, :].bitcast(F32))
                for g in range(2):
                    ps = tps.tile([128, 4, 128], F32)
                    for i in range(4):
                        et = g * 4 + i
                        nc.tensor.transpose(
                            ps[:, i, :], xs[:, et * 128 : (et + 1) * 128], ident
                        )
                    nc.vector.tensor_copy(
                        xt[:, g * 4 : (g + 1) * 4, st * 128 : (st + 1) * 128], ps
                    )

        # ---- phase A: V = x @ Wv + bv -> vdram ----
        with (
            tc.tile_pool(name="vps", bufs=4, space="PSUM") as vps,
            tc.tile_pool(name="vev", bufs=3) as vev,
        ):
            for st in range(ST):
                ps = vps.tile([128, E], F32)
                for half in range(2):
                    for et in range(ET):
                        nc.tensor.matmul(
                            ps[:, half * 512 : (half + 1) * 512],
                            xt[:, et, st * 128 : (st + 1) * 128],
                            wv[:, et, half * 512 : (half + 1) * 512],
                            start=(et == 0),
                            stop=(et == ET - 1),
                        )
                vb = vev.tile([128, E], F32R)
                nc.vector.tensor_add(vb, ps, bv_t)
                nc.sync.dma_start(out=vdram[st * 128 : (st + 1) * 128, :], in_=vb)
        wv_ctx.__exit__(None, None, None)

        # ---- phase B: per-pair QK JIT + attention, software-pipelined ----
        aout_pool = ctx.enter_context(tc.tile_pool(name="aout", bufs=1))
        aout = aout_pool.tile([128, NP, Q], F32R)

        with (
            tc.tile_pool(name="wqk", bufs=2) as wqkp,
            tc.tile_pool(name="qt", bufs=2) as qtp,
            tc.tile_pool(name="kt", bufs=2) as ktp,
            tc.tile_pool(name="vp", bufs=2) as vpp,
            tc.tile_pool(name="pt", bufs=3) as ptp,
            tc.tile_pool(name="ev", bufs=2) as evp,
            tc.tile_pool(name="qkps", bufs=1, space="PSUM") as qkps,
            tc.tile_pool(name="scps", bufs=2, space="PSUM") as scps,
            tc.tile_pool(name="accps", bufs=2, space="PSUM") as accps,
        ):

            def build_pair(p):
                """Allocate pair-p input tiles; return (tiles, emission thunks)."""
                wq = wqkp.tile([128, ET, 128], F32R, tag="wq")
                wk = wqkp.tile([128, ET, 128], F32R, tag="wk")
                qt_t = qtp.tile([128, Q], F32R)
                kt_t = ktp.tile([128, S], F32R)
                vp = vpp.tile([128, KT, 2, 65], F32R)
                th = []
                th.append(lambda: nc.sync.dma_start(
                    out=wq,
                    in_=wqkv[:, p * 128 : (p + 1) * 128].rearrange(
                        "(t p2) m -> p2 t m", p2=128),
                ))
                th.append(lambda: nc.sync.dma_start(
                    out=wk,
                    in_=wqkv[:, E + p * 128 : E + (p + 1) * 128].rearrange(
                        "(t p2) m -> p2 t m", p2=128),
                ))
                for h in range(2):
                    th.append(lambda h=h: nc.sync.dma_start(
                        out=vp[:, :, h, 0:64],
                        in_=vdram[
                            :, p * 128 + h * 64 : p * 128 + h * 64 + 64
                        ].rearrange("(t p2) d -> p2 t d", p2=128),
                    ))
                th.append(lambda: nc.sync.dma_start(
                    out=vp[:, :, :, 64:65], in_=_bcast_free(ones_r[:, 0:1], [KT * 2])
                ))

                def qk_group(dst, w, bias_col, xoff):
                    g = []
                    ps_box = []

                    def alloc():
                        qk_ps = qkps.tile([128, 1024], F32, tag="qk")
                        ps_box.append(qk_ps)
                    g.append(alloc)
                    for half in range(2):
                        for et in range(ET):
                            g.append(lambda half=half, et=et: nc.tensor.matmul(
                                ps_box[0][:, half * 512 : (half + 1) * 512],
                                w[:, et, :],
                                xt[:, et, xoff + half * 512 : xoff + (half + 1) * 512],
                                start=(et == 0),
                                stop=(et == ET - 1),
                            ))
                    g.append(lambda: nc.vector.tensor_scalar_add(dst, ps_box[0], bias_col))
                    return g

                th += qk_group(qt_t, wq, bqk_t[:, p : p + 1], 0)
                th += qk_group(kt_t[:, 0:1024], wk, bqk_t[:, 8 + p : 9 + p], 0)
                th += qk_group(kt_t[:, 1024:2048], wk, bqk_t[:, 8 + p : 9 + p], 1024)
                return {"qt": qt_t, "kt": kt_t, "vp": vp}, th

            cur, th0 = build_pair(0)
            for t in th0:
                t()

            for p in range(NP):
                if p + 1 < NP:
                    nxt, pending = build_pair(p + 1)
                else:
                    nxt = None
                    pending = []
                pending = list(pending)
                qt_t, kt_t, vp = cur["qt"], cur["kt"], cur["vp"]
                for qh in range(2):
                    qsl = slice(qh * 512, (qh + 1) * 512)
                    acc0 = accps.tile([128, 512], F32, tag="acc")
                    acc1 = accps.tile([128, 512], F32, tag="acc")
                    pts = [None] * KT
                    for k in range(KT):
                        sc = scps.tile([128, 1024], F32, tag="sc")
                        nc.tensor.matmul(
                            sc[:, 0:512],
                            kt_t[0:64, k * 128 : (k + 1) * 128],
                            qt_t[0:64, qsl],
                            start=True, stop=True,
                        )
                        nc.tensor.matmul(
                            sc[:, 512:1024],
                            kt_t[64:128, k * 128 : (k + 1) * 128],
                            qt_t[64:128, qsl],
                            start=True, stop=True,
                        )
                        if k >= 1:
                            pt_p = pts[k - 1]
                            nc.tensor.matmul(
                                acc0[0:65, :], vp[:, k - 1, 0, :], pt_p[:, 0:512],
                                start=(k - 1 == 0), stop=(k - 1 == KT - 1),
                            )
                            nc.tensor.matmul(
                                acc1[0:65, :], vp[:, k - 1, 1, :], pt_p[:, 512:1024],
                                start=(k - 1 == 0), stop=(k - 1 == KT - 1),
                            )
                        pt_t = ptp.tile([128, 1024], F32R)
                        pts[k] = pt_t
                        nc.scalar.activation(out=pt_t, in_=sc, func=AF.Exp, scale=0.125)
                        for _ in range(2):
                            if pending:
                                pending.pop(0)()
                    nc.tensor.matmul(
                        acc0[0:65, :], vp[:, KT - 1, 0, :], pts[KT - 1][:, 0:512],
                        start=False, stop=True,
                    )
                    nc.tensor.matmul(
                        acc1[0:65, :], vp[:, KT - 1, 1, :], pts[KT - 1][:, 512:1024],
                        start=False, stop=True,
                    )
                    # eviction: fast psum release, then off-path normalization
                    ridx = p * 2 + qh
                    au0 = evp.tile([128, 512], F32, tag="au0")
                    nc.vector.tensor_copy(au0[0:65, :], acc0[0:65, :])
                    au1 = evp.tile([128, 512], F32, tag="au1")
                    nc.vector.tensor_copy(au1[0:65, :], acc1[0:65, :])
                    nc.sync.dma_start(out=rscr[ridx : ridx + 1, 0:512], in_=au0[64:65, :])
                    nc.sync.dma_start(out=rscr[ridx : ridx + 1, 512:1024], in_=au1[64:65, :])
                    rw = evp.tile([64, 16], F32, tag="rw")
                    nc.sync.dma_start(
                        out=rw, in_=rscr[ridx : ridx + 1, :].rearrange("o (p f) -> (o p) f", p=64)
                    )
                    rwr = evp.tile([64, 16], F32, tag="rwr")
                    nc.vector.reciprocal(rwr, rw)
                    nc.sync.dma_start(
                        out=rscr2[ridx : ridx + 1, :].rearrange("o (p f) -> (o p) f", p=64),
                        in_=rwr,
                    )
                    sc0 = evp.tile([64, 512], F32, tag="sc0")
                    nc.sync.dma_start(out=sc0, in_=_bcast_dram(rscr2[ridx, 0:1], 64, 512))
                    sc1 = evp.tile([64, 512], F32, tag="sc1")
                    nc.sync.dma_start(out=sc1, in_=_bcast_dram(rscr2[ridx, 512:513], 64, 512))
                    nc.vector.tensor_mul(aout[0:64, p, qsl], au0[0:64, :], sc0)
                    tmp1 = evp.tile([64, 512], F32R, tag="tmp1")
                    nc.vector.tensor_mul(tmp1, au1[0:64, :], sc1)
                    nc.sync.dma_start(out=aout[64:128, p, qsl], in_=tmp1)
                for t in pending:
                    t()
                cur = nxt

        # ---- phase C: y = attn_out @ W_out + b_out ----
        with (
            tc.tile_pool(name="wo", bufs=2) as wop,
            tc.tile_pool(name="yps", bufs=4, space="PSUM") as yps,
            tc.tile_pool(name="yev", bufs=3) as yev,
        ):
            wo = [wop.tile([128, ET, 512], F32R, name=f"wo{h}", tag=f"wo{h}") for h in range(2)]
            for half in range(2):
                nc.sync.dma_start(
                    out=wo[half],
                    in_=wout[:, half * 512 : (half + 1) * 512].rearrange(
                        "(t p) n -> p t n", p=128
                    ),
                )
            for half in range(2):
                for qt_i in range(8):
                    ps = yps.tile([128, 512], F32)
                    for p8 in range(8):
                        nc.tensor.matmul(
                            ps,
                            aout[:, p8, qt_i * 128 : (qt_i + 1) * 128],
                            wo[half][:, p8, :],
                            start=(p8 == 0),
                            stop=(p8 == 7),
                        )
                    yb = yev.tile([128, 512], F32)
                    nc.vector.tensor_add(
                        yb, ps, bout_t[:, half * 512 : (half + 1) * 512]
                    )
                    nc.sync.dma_start(
                        out=y[qt_i * 128 : (qt_i + 1) * 128, half * 512 : (half + 1) * 512],
                        in_=yb,
                    )


def build_nc():
    nc = bacc.Bacc("TRN2", target_bir_lowering=False, debug=False)
    x = nc.dram_tensor("x", [S, E], F32R, kind="ExternalInput").ap()
    wqkv = nc.dram_tensor("wqkv", [E, 3 * E], F32R, kind="ExternalInput").ap()
    bqkv = nc.dram_tensor("bqkv", [3 * E], F32R, kind="ExternalInput").ap()
    wout = nc.dram_tensor("wout", [E, E], F32R, kind="ExternalInput").ap()
    bout = nc.dram_tensor("bout", [E], F32, kind="ExternalInput").ap()
    y = nc.dram_tensor("y", [Q, E], F32, kind="ExternalOutput").ap()
    vdram = nc.dram_tensor("vdram", [S, E], F32R).ap()
    rscr = nc.dram_tensor("rscr", [16, 1024], F32).ap()
    rscr2 = nc.dram_tensor("rscr2", [16, 1024], F32).ap()
    with tile.TileContext(nc) as tc:
        _emit(tc, nc, x, wqkv, bqkv, wout, bout, y, vdram, rscr, rscr2)
    nc.compile()
    return nc


_NC = None


def _get_nc():
    global _NC
    if _NC is None:
        _NC = build_nc()
    return _NC


def make_in_maps(x, W_qkv, b_qkv, W_out, b_out):
    x = np.ascontiguousarray(np.asarray(x, dtype=np.float32))
    W_qkv = np.ascontiguousarray(np.asarray(W_qkv, dtype=np.float32))
    b_qkv = np.ascontiguousarray(np.asarray(b_qkv, dtype=np.float32))
    W_out = np.ascontiguousarray(np.asarray(W_out, dtype=np.float32))
    b_out = np.ascontiguousarray(np.asarray(b_out, dtype=np.float32))
    in_maps = []
    for c in range(N_CORES):
        b, hq = c // 2, c % 2
        xb = x[b]
        if hq:
            xb = np.ascontiguousarray(np.concatenate([xb[1024:], xb[:1024]], axis=0))
        in_maps.append(
            {"x": xb, "wqkv": W_qkv, "bqkv": b_qkv, "wout": W_out, "bout": b_out}
        )
    return in_maps


def assemble(results):
    out = np.empty((B, S, E), dtype=np.float32)
    for c in range(N_CORES):
        b, hq = c // 2, c % 2
        out[b, hq * 1024 : (hq + 1) * 1024, :] = results[c]["y"]
    return out


def kernel(x, W_qkv, b_qkv, W_out, b_out):
    nc = _get_nc()
    in_maps = make_in_maps(x, W_qkv, b_qkv, W_out, b_out)
    res = run_bass_kernel_spmd(nc, in_maps, list(range(N_CORES)))
    return assemble(res.results)
